# revision 4
# baseline (speedup 1.0000x reference)
"""Trainium2 Bass kernel for nn_ConditionedConvolution2D.

Reference computation:
    A  = P @ dense_w                      # [B, 3*3*C*C_OUT] per-sample conv kernels
    Wk = A.reshape(B, 3, 3, C, C_OUT)
    Y[b] = conv2d(X[b], Wk[b])            # SAME padding, stride 1, NHWC

Strategy (pure data parallel, 4 samples per core on 8 cores):
  - im2col with contraction q=(dw,ci)=96: for each padded image row hp a
    single matmul with stationary lhsT [96, 128 w] and moving weights
    [96, 96=(dh_rev,co)] accumulates 3 output-row chunks of a PSUM bank
    ([128 w, 16 rows * 32 co]).
  - HBM traffic cut: only blocks dw=1 (center) and dw=2 are shipped from
    HBM ([64, S] per sample, port-balanced to SBUF partitions 32..95).
    The dw=0 block is the center data globally shifted by +1 element and
    is built on-device by quarter-granular shifted copies spread across
    DVE / ACT / GPSIMD into partitions 0..31.
  - 3 DMA rings: sync(qSP) + scalar(qAct) carry dense_w (interleaved
    quarters first) then slab quarters; gpsimd(SWDGE) carries the batched
    output stores ([128, 2048] = 4 PSUM banks per store).
  - Hypernetwork runs in 4 chunks of 24 g-columns as dense_w quarters
    arrive, hidden under the first slab loads.
  - Completed banks are drained PSUM->SBUF with bf16 cast alternating
    DVE/ACT into a [128, 2048] osb, then stored to a [b, w, h*co] DRAM
    layout; the host transposes back to NHWC and upcasts.
"""

import os
import sys

sys.path.insert(0, "/opt/trn_rl_repo")

import numpy as np
import ml_dtypes

import concourse.bacc as bacc
import concourse.mybir as mybir
import concourse.tile as tile
from concourse.bass_utils import run_bass_kernel_spmd

B, H, W, C = 32, 128, 128, 32
P_DIM = 128
KH = KW = 3
C_OUT = 32
N_CORES = 8
BPC = B // N_CORES          # samples per core
H2 = H + 2                  # padded rows
W2 = W + 4                  # padded row pitch (2 pad cols + 2 alignment)
S = H2 * W2                 # slab free size per partition
QK = KW * C                 # 96 contraction size (dw, ci)
G = KH * C_OUT              # 96 weight-stream columns per sample (dh_rev, co)
RPT = 16                    # output rows per PSUM tile (one full bank)

# slab DMA / copy quarters (row-aligned)
QROWS = [(0, 33), (33, 66), (66, 98), (98, H2)]
QRANGE = [(r0 * W2, r1 * W2) for r0, r1 in QROWS]

_NC_CACHE = {}


def _build_nc():
    f32 = mybir.dt.float32
    bf16 = mybir.dt.bfloat16
    nc = bacc.Bacc("TRN2", target_bir_lowering=False, debug=False,
                   num_devices=N_CORES)
    x_trip = nc.dram_tensor("x_trip", [BPC, QK, S], bf16,
                            kind="ExternalInput")
    p_t = nc.dram_tensor("p_t", [P_DIM, BPC], bf16, kind="ExternalInput")
    dw_t = nc.dram_tensor("dw_t", [P_DIM, KH * KW * C * C_OUT], bf16,
                          kind="ExternalInput")
    y = nc.dram_tensor("y", [BPC, W, H * C_OUT], bf16, kind="ExternalOutput")

    NDW = KH * KW * C * C_OUT        # 9216 dense_w columns
    DWQ = NDW // 4                   # dense_w quarter (24 g-groups)

    with tile.TileContext(nc) as tc:
        with tc.tile_pool(name="const", bufs=1) as cpool, \
             tc.tile_pool(name="wsb", bufs=1) as wsb_pool, \
             tc.tile_pool(name="slab", bufs=BPC) as slab_pool, \
             tc.tile_pool(name="osb", bufs=3) as osb_pool:

            # ---- inputs: p + dense_w quarters interleaved on both rings ----
            p_sb = cpool.tile([P_DIM, BPC], bf16, name="p_sb", tag="p_sb")
            nc.sync.dma_start(out=p_sb[:], in_=p_t[:])
            dwsb = cpool.tile([P_DIM, NDW], bf16, name="dwsb", tag="dwsb")
            for q in range(4):
                eng = nc.sync if q % 2 == 0 else nc.scalar
                eng.dma_start(out=dwsb[:, q * DWQ:(q + 1) * DWQ],
                              in_=dw_t[:, q * DWQ:(q + 1) * DWQ])

            # ---- slab loads: blocks dw1,dw2 from HBM into parts 32..96 ----
            slabs = []
            for b in range(BPC):
                slab = slab_pool.tile([QK, S], bf16, name=f"slab{b}",
                                      tag="slab")
                slabs.append(slab)
            for b in range(BPC):
                for q in range(4):
                    lo, hi = QRANGE[q]
                    eng = nc.sync if q % 2 == 0 else nc.scalar
                    eng.dma_start(out=slabs[b][32:96, lo:hi],
                                  in_=x_trip[b][32:96, lo:hi])

            # on-device dw0 block: center globally shifted +1 element.
            # quarter q copies src [32:64, lo:hi) -> dst [0:32, lo+1:hi+1)
            def emit_dw0_copies(b):
                slab = slabs[b]
                nc.gpsimd.memset(slab[0:32, 0:1], 0.0)
                for q in range(4):
                    lo, hi = QRANGE[q]
                    if q == 3:
                        hi -= 2          # keep count even; tail never read
                    src = slab[32:64, lo:hi]
                    dst = slab[0:32, lo + 1:hi + 1]
                    if q == 1:
                        nc.scalar.copy(out=dst, in_=src)
                    elif q == 3:
                        nc.gpsimd.tensor_copy(out=dst, in_=src)
                    else:
                        nc.vector.tensor_copy(out=dst, in_=src)

            emit_dw0_copies(0)

            # ---- hypernetwork: Wk = P @ dense_w (permuted), 4 chunks ----
            w_sb = wsb_pool.tile([QK, BPC * G], bf16, name="w_sb", tag="w_sb")
            with tc.tile_pool(name="wps", bufs=2, space="PSUM") as wps_pool:
                for chunk in range(4):
                    wps = wps_pool.tile([QK, 24 * BPC], f32, name="wps",
                                        tag="wps")
                    for gg in range(24):
                        g = chunk * 24 + gg      # g = dh_rev*C_OUT + co
                        nc.tensor.matmul(
                            out=wps[:, gg * BPC:(gg + 1) * BPC],
                            lhsT=dwsb[:, g * QK:(g + 1) * QK],
                            rhs=p_sb[:],
                            start=True, stop=True,
                        )
                    # permute (g, b) -> (b, g) while casting f32 -> bf16
                    src = wps[:].rearrange("p (g b) -> p g b", b=BPC)
                    dst = w_sb[:].rearrange("p (b g) -> p g b", g=G)[
                        :, chunk * 24:(chunk + 1) * 24, :]
                    nc.vector.tensor_copy(out=dst, in_=src)

            # ---- per-sample conv ----
            with tc.tile_pool(name="acc", bufs=4, space="PSUM") as acc_pool:
                for b in range(BPC):
                    if b + 1 < BPC:
                        emit_dw0_copies(b + 1)
                    slab = slabs[b]
                    osb = None
                    tiles = {}      # t -> psum AP [W, RPT*C_OUT]
                    for hp in range(H2):
                        lhsT = slab[:, hp * W2: hp * W2 + W]
                        rows = [r for r in (hp - 2, hp - 1, hp)
                                if 0 <= r < H]
                        groups = []
                        for r in rows:
                            t = r // RPT
                            if groups and groups[-1][0] == t:
                                groups[-1][1].append(r)
                            else:
                                groups.append((t, [r]))
                        for t, rs in groups:
                            if t not in tiles:
                                tiles[t] = acc_pool.tile(
                                    [W, RPT * C_OUT], f32, name="acc",
                                    tag="acc")
                            r_lo, r_hi = rs[0], rs[-1]
                            c_lo = r_lo % RPT
                            w_lo = 2 - (hp - r_lo)
                            nc.tensor.matmul(
                                out=tiles[t][:, c_lo * C_OUT:
                                             (c_lo + len(rs)) * C_OUT],
                                lhsT=lhsT,
                                rhs=w_sb[:, b * G + w_lo * C_OUT:
                                         b * G + (w_lo + len(rs)) * C_OUT],
                                start=(r_lo % RPT == 0 and hp - r_lo == 0),
                                stop=(r_hi % RPT == RPT - 1
                                      and hp - r_hi == 2),
                                skip_group_check=True,
                            )
                        t_done = None
                        if hp >= 2 and (hp - 2) % RPT == RPT - 1:
                            t_done = (hp - 2) // RPT
                        if t_done is not None:
                            gpos = t_done % 4
                            if gpos == 0:
                                osb = osb_pool.tile([W, 4 * RPT * C_OUT],
                                                    bf16, name="osb",
                                                    tag="osb")
                            src = tiles.pop(t_done)
                            dst = osb[:, gpos * RPT * C_OUT:
                                      (gpos + 1) * RPT * C_OUT]
                            if t_done % 2 == 0:
                                nc.vector.tensor_copy(out=dst, in_=src[:])
                            else:
                                nc.scalar.copy(out=dst, in_=src[:])
                            if gpos == 3:
                                g0 = (t_done - 3) * RPT * C_OUT
                                nc.gpsimd.dma_start(
                                    out=y[b][:, g0:g0 + 4 * RPT * C_OUT],
                                    in_=osb[:],
                                )
    nc.finalize()
    return nc


def _get_nc():
    if "nc" not in _NC_CACHE:
        _NC_CACHE["nc"] = _build_nc()
    return _NC_CACHE["nc"]


def _prep_inputs(X, P, dense_w):
    bf16 = ml_dtypes.bfloat16
    Xb = np.ascontiguousarray(X.transpose(0, 3, 1, 2)).astype(bf16)  # [B,C,H,W]
    X_trip = np.zeros((B, QK, H2, W2), dtype=bf16)
    for dw in range(KW):
        lo = max(0, 1 - dw)          # first valid wp
        hi = W - dw                  # last valid wp (inclusive)
        src_lo = lo + dw - 1
        X_trip[:, dw * C:(dw + 1) * C, 1:H + 1, lo:hi + 1] = \
            Xb[:, :, :, src_lo:W]
    X_trip = X_trip.reshape(B, QK, S)

    # dense_w columns j = ((dh*3+dw)*C+ci)*C_OUT+co -> (2-dh, co, dw, ci)
    dwp = np.ascontiguousarray(
        dense_w.reshape(P_DIM, KH, KW, C, C_OUT)[:, ::-1]
        .transpose(0, 1, 4, 2, 3)
        .reshape(P_DIM, -1)
    ).astype(bf16)

    in_maps = []
    for c in range(N_CORES):
        sl = slice(c * BPC, (c + 1) * BPC)
        in_maps.append({
            "x_trip": np.ascontiguousarray(X_trip[sl]),
            "p_t": np.ascontiguousarray(P[sl].T).astype(bf16),
            "dw_t": dwp,
        })
    return in_maps


def _run(X, P, dense_w, **spmd_kwargs):
    nc = _get_nc()
    in_maps = _prep_inputs(X, P, dense_w)
    res = run_bass_kernel_spmd(nc, in_maps, core_ids=list(range(N_CORES)),
                               **spmd_kwargs)
    outs = []
    for c in range(N_CORES):
        yv = res.results[c]["y"].astype(np.float32)
        yv = yv.reshape(BPC, W, H, C_OUT)
        outs.append(yv.transpose(0, 2, 1, 3))        # -> [b, h, w, co]
    Y = np.ascontiguousarray(np.concatenate(outs, axis=0), dtype=np.float32)
    return Y, res


def kernel(X, P, dense_w):
    Y, _ = _run(np.asarray(X), np.asarray(P), np.asarray(dense_w))
    return Y


# revision 7
# speedup vs baseline: 1.5954x; 1.5954x over previous
"""Trainium2 Bass kernel for nn_ConditionedConvolution2D.

Reference computation:
    A  = P @ dense_w                      # [B, 3*3*C*C_OUT] per-sample conv kernels
    Wk = A.reshape(B, 3, 3, C, C_OUT)
    Y[b] = conv2d(X[b], Wk[b])            # SAME padding, stride 1, NHWC

Strategy (pure data parallel, 4 samples per core on 8 cores):
  - im2col with contraction q=(dw,ci)=96: for each padded image row hp a
    single matmul with stationary lhsT [96, 128 w] and moving weights
    [96, 96=(dh_rev,co)] accumulates 3 output-row chunks of a PSUM bank
    ([128 w, 16 rows * 32 co]).
  - HBM traffic cut: only blocks dw=1 (center) and dw=2 are shipped from
    HBM ([64, S] per sample, port-balanced to SBUF partitions 32..95).
    The dw=0 block is the center data globally shifted by +1 element and
    is built on-device by quarter-granular shifted copies spread across
    DVE / ACT / GPSIMD into partitions 0..31.
  - 3 DMA rings: sync(qSP) + scalar(qAct) carry dense_w (interleaved
    quarters first) then slab quarters; gpsimd(SWDGE) carries the batched
    output stores ([128, 2048] = 4 PSUM banks per store).
  - Hypernetwork runs in 4 chunks of 24 g-columns as dense_w quarters
    arrive, hidden under the first slab loads.
  - Completed banks are drained PSUM->SBUF with bf16 cast alternating
    DVE/ACT into a [128, 2048] osb, then stored to a [b, w, h*co] DRAM
    layout; the host transposes back to NHWC and upcasts.
"""

import os
import sys

sys.path.insert(0, "/opt/trn_rl_repo")

import numpy as np
import ml_dtypes

import concourse.bacc as bacc
import concourse.mybir as mybir
import concourse.tile as tile
from concourse.bass_utils import run_bass_kernel_spmd

B, H, W, C = 32, 128, 128, 32
P_DIM = 128
KH = KW = 3
C_OUT = 32
N_CORES = 8
BPC = B // N_CORES          # samples per core
H2 = H + 2                  # padded rows
W2 = W + 4                  # padded row pitch (2 pad cols + 2 alignment)
S = H2 * W2                 # slab free size per partition
QK = KW * C                 # 96 contraction size (dw, ci)
G = KH * C_OUT              # 96 weight-stream columns per sample (dh_rev, co)
RPT = 16                    # output rows per PSUM tile (one full bank)

# slab DMA / copy quarters (row-aligned)
QROWS = [(0, 33), (33, 66), (66, 98), (98, H2)]
QRANGE = [(r0 * W2, r1 * W2) for r0, r1 in QROWS]

_NC_CACHE = {}


def _build_nc():
    f32 = mybir.dt.float32
    bf16 = mybir.dt.bfloat16
    nc = bacc.Bacc("TRN2", target_bir_lowering=False, debug=False,
                   num_devices=N_CORES)
    x_trip = nc.dram_tensor("x_trip", [BPC, QK, S], bf16,
                            kind="ExternalInput")
    p_t = nc.dram_tensor("p_t", [P_DIM, BPC], bf16, kind="ExternalInput")
    dw_t = nc.dram_tensor("dw_t", [P_DIM, KH * KW * C * C_OUT], bf16,
                          kind="ExternalInput")
    y = nc.dram_tensor("y", [BPC, W, H * C_OUT], bf16, kind="ExternalOutput")

    NDW = KH * KW * C * C_OUT        # 9216 dense_w columns
    DWQ = NDW // 4                   # dense_w quarter (24 g-groups)

    with tile.TileContext(nc) as tc:
        with tc.tile_pool(name="const", bufs=1) as cpool, \
             tc.tile_pool(name="wsb", bufs=1) as wsb_pool, \
             tc.tile_pool(name="slab", bufs=BPC) as slab_pool, \
             tc.tile_pool(name="osb", bufs=3) as osb_pool:

            # ---- inputs: p + dense_w quarters interleaved on both rings ----
            p_sb = cpool.tile([P_DIM, BPC], bf16, name="p_sb", tag="p_sb")
            nc.sync.dma_start(out=p_sb[:], in_=p_t[:])
            dwsb = cpool.tile([P_DIM, NDW], bf16, name="dwsb", tag="dwsb")
            for q in range(4):
                eng = nc.sync if q % 2 == 0 else nc.scalar
                eng.dma_start(out=dwsb[:, q * DWQ:(q + 1) * DWQ],
                              in_=dw_t[:, q * DWQ:(q + 1) * DWQ])

            # ---- slab loads: blocks dw1,dw2 from HBM into parts 32..96 ----
            slabs = []
            for b in range(BPC):
                slab = slab_pool.tile([QK, S], bf16, name=f"slab{b}",
                                      tag="slab")
                slabs.append(slab)
            for b in range(BPC):
                for q in range(4):
                    lo, hi = QRANGE[q]
                    eng = nc.sync if q % 2 == 0 else nc.scalar
                    eng.dma_start(out=slabs[b][32:96, lo:hi],
                                  in_=x_trip[b][32:96, lo:hi])

            # on-device dw0 block: center globally shifted +1 element.
            # quarter q copies src [32:64, lo:hi) -> dst [0:32, lo+1:hi+1)
            # DVE takes 3 quarters (hits 4x perf mode), ACT takes 1 (1x).
            def emit_dw0_quarter(b, q):
                slab = slabs[b]
                lo, hi = QRANGE[q]
                if q == 3:
                    hi -= 2              # keep count even; tail never read
                src = slab[32:64, lo:hi]
                dst = slab[0:32, lo + 1:hi + 1]
                if q == 3:
                    nc.scalar.copy(out=dst, in_=src)
                else:
                    nc.vector.tensor_copy(out=dst, in_=src)

            nc.gpsimd.memset(slabs[0][0:32, 0:1], 0.0)
            for q in range(4):
                emit_dw0_quarter(0, q)

            # ---- hypernetwork: Wk = P @ dense_w (permuted), 4 chunks ----
            w_sb = wsb_pool.tile([QK, BPC * G], bf16, name="w_sb", tag="w_sb")
            with tc.tile_pool(name="wps", bufs=2, space="PSUM") as wps_pool:
                for chunk in range(4):
                    wps = wps_pool.tile([QK, 24 * BPC], f32, name="wps",
                                        tag="wps")
                    for gg in range(24):
                        g = chunk * 24 + gg      # g = dh_rev*C_OUT + co
                        nc.tensor.matmul(
                            out=wps[:, gg * BPC:(gg + 1) * BPC],
                            lhsT=dwsb[:, g * QK:(g + 1) * QK],
                            rhs=p_sb[:],
                            start=True, stop=True,
                        )
                    # permute (g, b) -> (b, g) while casting f32 -> bf16
                    src = wps[:].rearrange("p (g b) -> p g b", b=BPC)
                    dst = w_sb[:].rearrange("p (b g) -> p g b", g=G)[
                        :, chunk * 24:(chunk + 1) * 24, :]
                    nc.vector.tensor_copy(out=dst, in_=src)

            # ---- per-sample conv ----
            with tc.tile_pool(name="acc", bufs=4, space="PSUM") as acc_pool:
                for b in range(BPC):
                    if b + 1 < BPC:
                        nc.gpsimd.memset(slabs[b + 1][0:32, 0:1], 0.0)
                    copies_todo = list(range(4)) if b + 1 < BPC else []
                    slab = slabs[b]
                    osb = None
                    tiles = {}      # t -> psum AP [W, RPT*C_OUT]
                    for hp in range(H2):
                        lhsT = slab[:, hp * W2: hp * W2 + W]
                        rows = [r for r in (hp - 2, hp - 1, hp)
                                if 0 <= r < H]
                        groups = []
                        for r in rows:
                            t = r // RPT
                            if groups and groups[-1][0] == t:
                                groups[-1][1].append(r)
                            else:
                                groups.append((t, [r]))
                        for t, rs in groups:
                            if t not in tiles:
                                tiles[t] = acc_pool.tile(
                                    [W, RPT * C_OUT], f32, name="acc",
                                    tag="acc")
                            r_lo, r_hi = rs[0], rs[-1]
                            c_lo = r_lo % RPT
                            w_lo = 2 - (hp - r_lo)
                            nc.tensor.matmul(
                                out=tiles[t][:, c_lo * C_OUT:
                                             (c_lo + len(rs)) * C_OUT],
                                lhsT=lhsT,
                                rhs=w_sb[:, b * G + w_lo * C_OUT:
                                         b * G + (w_lo + len(rs)) * C_OUT],
                                start=(r_lo % RPT == 0 and hp - r_lo == 0),
                                stop=(r_hi % RPT == RPT - 1
                                      and hp - r_hi == 2),
                                skip_group_check=True,
                            )
                        t_done = None
                        if hp >= 2 and (hp - 2) % RPT == RPT - 1:
                            t_done = (hp - 2) // RPT
                        if t_done is not None:
                            gpos = t_done % 4
                            if gpos == 0:
                                osb = osb_pool.tile([W, 4 * RPT * C_OUT],
                                                    bf16, name="osb",
                                                    tag="osb")
                            src = tiles.pop(t_done)
                            dst = osb[:, gpos * RPT * C_OUT:
                                      (gpos + 1) * RPT * C_OUT]
                            if t_done % 2 == 0:
                                nc.vector.tensor_copy(out=dst, in_=src[:])
                            else:
                                nc.scalar.copy(out=dst, in_=src[:])
                            # interleave next sample's dw0 copy quarters
                            # between drains so no engine FIFO blocks long
                            if copies_todo and t_done in (1, 3, 5, 6):
                                emit_dw0_quarter(b + 1, copies_todo.pop(0))
                            if gpos == 3:
                                g0 = (t_done - 3) * RPT * C_OUT
                                nc.gpsimd.dma_start(
                                    out=y[b][:, g0:g0 + 4 * RPT * C_OUT],
                                    in_=osb[:],
                                )
    nc.finalize()
    return nc


def _get_nc():
    if "nc" not in _NC_CACHE:
        _NC_CACHE["nc"] = _build_nc()
    return _NC_CACHE["nc"]


def _prep_inputs(X, P, dense_w):
    bf16 = ml_dtypes.bfloat16
    Xb = np.ascontiguousarray(X.transpose(0, 3, 1, 2)).astype(bf16)  # [B,C,H,W]
    X_trip = np.zeros((B, QK, H2, W2), dtype=bf16)
    for dw in range(KW):
        lo = max(0, 1 - dw)          # first valid wp
        hi = W - dw                  # last valid wp (inclusive)
        src_lo = lo + dw - 1
        X_trip[:, dw * C:(dw + 1) * C, 1:H + 1, lo:hi + 1] = \
            Xb[:, :, :, src_lo:W]
    X_trip = X_trip.reshape(B, QK, S)

    # dense_w columns j = ((dh*3+dw)*C+ci)*C_OUT+co -> (2-dh, co, dw, ci)
    dwp = np.ascontiguousarray(
        dense_w.reshape(P_DIM, KH, KW, C, C_OUT)[:, ::-1]
        .transpose(0, 1, 4, 2, 3)
        .reshape(P_DIM, -1)
    ).astype(bf16)

    in_maps = []
    for c in range(N_CORES):
        sl = slice(c * BPC, (c + 1) * BPC)
        in_maps.append({
            "x_trip": np.ascontiguousarray(X_trip[sl]),
            "p_t": np.ascontiguousarray(P[sl].T).astype(bf16),
            "dw_t": dwp,
        })
    return in_maps


def _run(X, P, dense_w, **spmd_kwargs):
    nc = _get_nc()
    in_maps = _prep_inputs(X, P, dense_w)
    res = run_bass_kernel_spmd(nc, in_maps, core_ids=list(range(N_CORES)),
                               **spmd_kwargs)
    outs = []
    for c in range(N_CORES):
        yv = res.results[c]["y"].astype(np.float32)
        yv = yv.reshape(BPC, W, H, C_OUT)
        outs.append(yv.transpose(0, 2, 1, 3))        # -> [b, h, w, co]
    Y = np.ascontiguousarray(np.concatenate(outs, axis=0), dtype=np.float32)
    return Y, res


def kernel(X, P, dense_w):
    Y, _ = _run(np.asarray(X), np.asarray(P), np.asarray(dense_w))
    return Y


# revision 8
# speedup vs baseline: 1.6262x; 1.0193x over previous
"""Trainium2 Bass kernel for nn_ConditionedConvolution2D.

Reference computation:
    A  = P @ dense_w                      # [B, 3*3*C*C_OUT] per-sample conv kernels
    Wk = A.reshape(B, 3, 3, C, C_OUT)
    Y[b] = conv2d(X[b], Wk[b])            # SAME padding, stride 1, NHWC

Strategy (pure data parallel, 4 samples per core on 8 cores):
  - im2col with contraction q=(dw,ci)=96: for each padded image row hp a
    single matmul with stationary lhsT [96, 128 w] and moving weights
    [96, 96=(dh_rev,co)] accumulates 3 output-row chunks of a PSUM bank
    ([128 w, 16 rows * 32 co]).
  - HBM traffic cut: only blocks dw=1 (center) and dw=2 are shipped from
    HBM ([64, S] per sample, port-balanced to SBUF partitions 32..95).
    The dw=0 block is the center data globally shifted by +1 element and
    is built on-device by quarter-granular shifted copies spread across
    DVE / ACT / GPSIMD into partitions 0..31.
  - 3 DMA rings: sync(qSP) + scalar(qAct) carry dense_w (interleaved
    quarters first) then slab quarters; gpsimd(SWDGE) carries the batched
    output stores ([128, 2048] = 4 PSUM banks per store).
  - Hypernetwork runs in 4 chunks of 24 g-columns as dense_w quarters
    arrive, hidden under the first slab loads.
  - Completed banks are drained PSUM->SBUF with bf16 cast alternating
    DVE/ACT into a [128, 2048] osb, then stored to a [b, w, h*co] DRAM
    layout; the host transposes back to NHWC and upcasts.
"""

import os
import sys

sys.path.insert(0, "/opt/trn_rl_repo")

import numpy as np
import ml_dtypes

import concourse.bacc as bacc
import concourse.mybir as mybir
import concourse.tile as tile
from concourse.bass_utils import run_bass_kernel_spmd

B, H, W, C = 32, 128, 128, 32
P_DIM = 128
KH = KW = 3
C_OUT = 32
N_CORES = 8
BPC = B // N_CORES          # samples per core
H2 = H + 2                  # padded rows
W2 = W + 4                  # padded row pitch (2 pad cols + 2 alignment)
S = H2 * W2                 # slab free size per partition
QK = KW * C                 # 96 contraction size (dw, ci)
G = KH * C_OUT              # 96 weight-stream columns per sample (dh_rev, co)
RPT = 16                    # output rows per PSUM tile (one full bank)

# slab DMA / copy quarters (row-aligned)
QROWS = [(0, 33), (33, 66), (66, 98), (98, H2)]
QRANGE = [(r0 * W2, r1 * W2) for r0, r1 in QROWS]

_NC_CACHE = {}


def _build_nc():
    f32 = mybir.dt.float32
    bf16 = mybir.dt.bfloat16
    nc = bacc.Bacc("TRN2", target_bir_lowering=False, debug=False,
                   num_devices=N_CORES)
    x_trip = nc.dram_tensor("x_trip", [BPC, QK, S], bf16,
                            kind="ExternalInput")
    p_t = nc.dram_tensor("p_t", [P_DIM, BPC], bf16, kind="ExternalInput")
    dw_t = nc.dram_tensor("dw_t", [P_DIM, KH * KW * C * C_OUT], bf16,
                          kind="ExternalInput")
    y = nc.dram_tensor("y", [BPC, W, H * C_OUT], bf16, kind="ExternalOutput")

    NDW = KH * KW * C * C_OUT        # 9216 dense_w columns
    DWQ = NDW // 4                   # dense_w quarter (24 g-groups)

    with tile.TileContext(nc) as tc:
        with tc.tile_pool(name="const", bufs=1) as cpool, \
             tc.tile_pool(name="wsb", bufs=1) as wsb_pool, \
             tc.tile_pool(name="slab", bufs=BPC) as slab_pool, \
             tc.tile_pool(name="osb", bufs=3) as osb_pool:

            # ---- inputs: p + dense_w quarters interleaved on both rings ----
            p_sb = cpool.tile([P_DIM, BPC], bf16, name="p_sb", tag="p_sb")
            nc.sync.dma_start(out=p_sb[:], in_=p_t[:])
            dwsb = cpool.tile([P_DIM, NDW], bf16, name="dwsb", tag="dwsb")
            for q in range(4):
                eng = nc.sync if q % 2 == 0 else nc.scalar
                eng.dma_start(out=dwsb[:, q * DWQ:(q + 1) * DWQ],
                              in_=dw_t[:, q * DWQ:(q + 1) * DWQ])

            # ---- slab loads: blocks dw1,dw2 from HBM into parts 32..96 ----
            slabs = []
            for b in range(BPC):
                slab = slab_pool.tile([QK, S], bf16, name=f"slab{b}",
                                      tag="slab")
                slabs.append(slab)
            for b in range(BPC):
                for q in range(4):
                    lo, hi = QRANGE[q]
                    eng = nc.sync if q % 2 == 0 else nc.scalar
                    eng.dma_start(out=slabs[b][32:96, lo:hi],
                                  in_=x_trip[b][32:96, lo:hi])

            # on-device dw0 block: center globally shifted +1 element.
            # quarter q copies src [32:64, lo:hi) -> dst [0:32, lo+1:hi+1)
            # DVE takes 3 quarters (hits 4x perf mode), ACT takes 1 (1x).
            def emit_dw0_quarter(b, q):
                slab = slabs[b]
                lo, hi = QRANGE[q]
                if q == 3:
                    hi -= 2              # keep count even; tail never read
                src = slab[32:64, lo:hi]
                dst = slab[0:32, lo + 1:hi + 1]
                if q == 3:
                    nc.scalar.copy(out=dst, in_=src)
                else:
                    nc.vector.tensor_copy(out=dst, in_=src)

            # ---- hypernetwork: Wk = P @ dense_w (permuted), 4 chunks ----
            w_sb = wsb_pool.tile([QK, BPC * G], bf16, name="w_sb", tag="w_sb")
            with tc.tile_pool(name="wps", bufs=4, space="PSUM") as wps_pool:
                for chunk in range(4):
                    wps = wps_pool.tile([QK, 24 * BPC], f32, name="wps",
                                        tag="wps")
                    for gg in range(24):
                        g = chunk * 24 + gg      # g = dh_rev*C_OUT + co
                        nc.tensor.matmul(
                            out=wps[:, gg * BPC:(gg + 1) * BPC],
                            lhsT=dwsb[:, g * QK:(g + 1) * QK],
                            rhs=p_sb[:],
                            start=True, stop=True,
                        )
                    # permute (g, b) -> (b, g) while casting f32 -> bf16
                    src = wps[:].rearrange("p (g b) -> p g b", b=BPC)
                    dst = w_sb[:].rearrange("p (b g) -> p g b", g=G)[
                        :, chunk * 24:(chunk + 1) * 24, :]
                    nc.vector.tensor_copy(out=dst, in_=src)

            nc.gpsimd.memset(slabs[0][0:32, 0:1], 0.0)
            for q in range(4):
                emit_dw0_quarter(0, q)

            # ---- per-sample conv ----
            with tc.tile_pool(name="acc", bufs=4, space="PSUM") as acc_pool:
                for b in range(BPC):
                    if b + 1 < BPC:
                        nc.gpsimd.memset(slabs[b + 1][0:32, 0:1], 0.0)
                    copies_todo = list(range(4)) if b + 1 < BPC else []
                    slab = slabs[b]
                    osb = None
                    tiles = {}      # t -> psum AP [W, RPT*C_OUT]
                    for hp in range(H2):
                        lhsT = slab[:, hp * W2: hp * W2 + W]
                        rows = [r for r in (hp - 2, hp - 1, hp)
                                if 0 <= r < H]
                        groups = []
                        for r in rows:
                            t = r // RPT
                            if groups and groups[-1][0] == t:
                                groups[-1][1].append(r)
                            else:
                                groups.append((t, [r]))
                        for t, rs in groups:
                            if t not in tiles:
                                tiles[t] = acc_pool.tile(
                                    [W, RPT * C_OUT], f32, name="acc",
                                    tag="acc")
                            r_lo, r_hi = rs[0], rs[-1]
                            c_lo = r_lo % RPT
                            w_lo = 2 - (hp - r_lo)
                            nc.tensor.matmul(
                                out=tiles[t][:, c_lo * C_OUT:
                                             (c_lo + len(rs)) * C_OUT],
                                lhsT=lhsT,
                                rhs=w_sb[:, b * G + w_lo * C_OUT:
                                         b * G + (w_lo + len(rs)) * C_OUT],
                                start=(r_lo % RPT == 0 and hp - r_lo == 0),
                                stop=(r_hi % RPT == RPT - 1
                                      and hp - r_hi == 2),
                                skip_group_check=True,
                            )
                        t_done = None
                        if hp >= 2 and (hp - 2) % RPT == RPT - 1:
                            t_done = (hp - 2) // RPT
                        if t_done is not None:
                            gpos = t_done % 4
                            if gpos == 0:
                                osb = osb_pool.tile([W, 4 * RPT * C_OUT],
                                                    bf16, name="osb",
                                                    tag="osb")
                            src = tiles.pop(t_done)
                            dst = osb[:, gpos * RPT * C_OUT:
                                      (gpos + 1) * RPT * C_OUT]
                            if t_done % 2 == 0:
                                nc.vector.tensor_copy(out=dst, in_=src[:])
                            else:
                                nc.scalar.copy(out=dst, in_=src[:])
                            # interleave next sample's dw0 copy quarters
                            # between drains so no engine FIFO blocks long
                            if copies_todo and t_done in (0, 1, 2, 3):
                                emit_dw0_quarter(b + 1, copies_todo.pop(0))
                            if b == BPC - 1 and gpos in (1, 3):
                                g0 = (t_done - 1) * RPT * C_OUT
                                o0 = (gpos - 1) * RPT * C_OUT
                                nc.gpsimd.dma_start(
                                    out=y[b][:, g0:g0 + 2 * RPT * C_OUT],
                                    in_=osb[:, o0:o0 + 2 * RPT * C_OUT],
                                )
                            elif b < BPC - 1 and gpos == 3:
                                g0 = (t_done - 3) * RPT * C_OUT
                                nc.gpsimd.dma_start(
                                    out=y[b][:, g0:g0 + 4 * RPT * C_OUT],
                                    in_=osb[:],
                                )
    nc.finalize()
    return nc


def _get_nc():
    if "nc" not in _NC_CACHE:
        _NC_CACHE["nc"] = _build_nc()
    return _NC_CACHE["nc"]


def _prep_inputs(X, P, dense_w):
    bf16 = ml_dtypes.bfloat16
    Xb = np.ascontiguousarray(X.transpose(0, 3, 1, 2)).astype(bf16)  # [B,C,H,W]
    X_trip = np.zeros((B, QK, H2, W2), dtype=bf16)
    for dw in range(KW):
        lo = max(0, 1 - dw)          # first valid wp
        hi = W - dw                  # last valid wp (inclusive)
        src_lo = lo + dw - 1
        X_trip[:, dw * C:(dw + 1) * C, 1:H + 1, lo:hi + 1] = \
            Xb[:, :, :, src_lo:W]
    X_trip = X_trip.reshape(B, QK, S)

    # dense_w columns j = ((dh*3+dw)*C+ci)*C_OUT+co -> (2-dh, co, dw, ci)
    dwp = np.ascontiguousarray(
        dense_w.reshape(P_DIM, KH, KW, C, C_OUT)[:, ::-1]
        .transpose(0, 1, 4, 2, 3)
        .reshape(P_DIM, -1)
    ).astype(bf16)

    in_maps = []
    for c in range(N_CORES):
        sl = slice(c * BPC, (c + 1) * BPC)
        in_maps.append({
            "x_trip": np.ascontiguousarray(X_trip[sl]),
            "p_t": np.ascontiguousarray(P[sl].T).astype(bf16),
            "dw_t": dwp,
        })
    return in_maps


def _run(X, P, dense_w, **spmd_kwargs):
    nc = _get_nc()
    in_maps = _prep_inputs(X, P, dense_w)
    res = run_bass_kernel_spmd(nc, in_maps, core_ids=list(range(N_CORES)),
                               **spmd_kwargs)
    outs = []
    for c in range(N_CORES):
        yv = res.results[c]["y"].astype(np.float32)
        yv = yv.reshape(BPC, W, H, C_OUT)
        outs.append(yv.transpose(0, 2, 1, 3))        # -> [b, h, w, co]
    Y = np.ascontiguousarray(np.concatenate(outs, axis=0), dtype=np.float32)
    return Y, res


def kernel(X, P, dense_w):
    Y, _ = _run(np.asarray(X), np.asarray(P), np.asarray(dense_w))
    return Y


# revision 9
# speedup vs baseline: 1.6548x; 1.0176x over previous
"""Trainium2 Bass kernel for nn_ConditionedConvolution2D.

Reference computation:
    A  = P @ dense_w                      # [B, 3*3*C*C_OUT] per-sample conv kernels
    Wk = A.reshape(B, 3, 3, C, C_OUT)
    Y[b] = conv2d(X[b], Wk[b])            # SAME padding, stride 1, NHWC

Strategy (pure data parallel, 4 samples per core on 8 cores):
  - im2col with contraction q=(dw,ci)=96: for each padded image row hp a
    single matmul with stationary lhsT [96, 128 w] and moving weights
    [96, 96=(dh_rev,co)] accumulates 3 output-row chunks of a PSUM bank
    ([128 w, 16 rows * 32 co]).
  - HBM traffic cut: only blocks dw=1 (center) and dw=2 are shipped from
    HBM ([64, S] per sample, port-balanced to SBUF partitions 32..95).
    The dw=0 block is the center data globally shifted by +1 element and
    is built on-device by quarter-granular shifted copies spread across
    DVE / ACT / GPSIMD into partitions 0..31.
  - 3 DMA rings: sync(qSP) + scalar(qAct) carry dense_w (interleaved
    quarters first) then slab quarters; gpsimd(SWDGE) carries the batched
    output stores ([128, 2048] = 4 PSUM banks per store).
  - Hypernetwork runs in 4 chunks of 24 g-columns as dense_w quarters
    arrive, hidden under the first slab loads.
  - Completed banks are drained PSUM->SBUF with bf16 cast alternating
    DVE/ACT into a [128, 2048] osb, then stored to a [b, w, h*co] DRAM
    layout; the host transposes back to NHWC and upcasts.
"""

import os
import sys

sys.path.insert(0, "/opt/trn_rl_repo")

import numpy as np
import ml_dtypes

import concourse.bacc as bacc
import concourse.mybir as mybir
import concourse.tile as tile
from concourse.bass_utils import run_bass_kernel_spmd

B, H, W, C = 32, 128, 128, 32
P_DIM = 128
KH = KW = 3
C_OUT = 32
N_CORES = 8
BPC = B // N_CORES          # samples per core
H2 = H + 2                  # padded rows
W2 = W + 4                  # padded row pitch (2 pad cols + 2 alignment)
S = H2 * W2                 # slab free size per partition
QK = KW * C                 # 96 contraction size (dw, ci)
G = KH * C_OUT              # 96 weight-stream columns per sample (dh_rev, co)
RPT = 16                    # output rows per PSUM tile (one full bank)

# slab DMA / copy quarters (row-aligned)
QROWS = [(0, 33), (33, 66), (66, 98), (98, H2)]
QRANGE = [(r0 * W2, r1 * W2) for r0, r1 in QROWS]

_NC_CACHE = {}


def _build_nc():
    f32 = mybir.dt.float32
    bf16 = mybir.dt.bfloat16
    nc = bacc.Bacc("TRN2", target_bir_lowering=False, debug=False,
                   num_devices=N_CORES)
    x_trip = nc.dram_tensor("x_trip", [BPC, QK, S], bf16,
                            kind="ExternalInput")
    p_t = nc.dram_tensor("p_t", [P_DIM, BPC], bf16, kind="ExternalInput")
    dw_t = nc.dram_tensor("dw_t", [P_DIM, KH * KW * C * C_OUT], bf16,
                          kind="ExternalInput")
    y = nc.dram_tensor("y", [BPC, W, H * C_OUT], bf16, kind="ExternalOutput")

    NDW = KH * KW * C * C_OUT        # 9216 dense_w columns
    DWQ = NDW // 4                   # dense_w quarter (24 g-groups)

    with tile.TileContext(nc) as tc:
        with tc.tile_pool(name="const", bufs=1) as cpool, \
             tc.tile_pool(name="wsb", bufs=1) as wsb_pool, \
             tc.tile_pool(name="slab", bufs=BPC) as slab_pool, \
             tc.tile_pool(name="osb", bufs=4) as osb_pool:

            # ---- inputs: p + dense_w quarters interleaved on both rings ----
            p_sb = cpool.tile([P_DIM, BPC], bf16, name="p_sb", tag="p_sb")
            nc.sync.dma_start(out=p_sb[:], in_=p_t[:])
            dwsb = cpool.tile([P_DIM, NDW], bf16, name="dwsb", tag="dwsb")
            for q in range(4):
                eng = nc.sync if q % 2 == 0 else nc.scalar
                eng.dma_start(out=dwsb[:, q * DWQ:(q + 1) * DWQ],
                              in_=dw_t[:, q * DWQ:(q + 1) * DWQ])

            # ---- slab loads: blocks dw1,dw2 from HBM into parts 32..96 ----
            slabs = []
            for b in range(BPC):
                slab = slab_pool.tile([QK, S], bf16, name=f"slab{b}",
                                      tag="slab")
                slabs.append(slab)
            for b in range(BPC):
                for q in range(4):
                    lo, hi = QRANGE[q]
                    eng = nc.sync if q % 2 == 0 else nc.scalar
                    eng.dma_start(out=slabs[b][32:96, lo:hi],
                                  in_=x_trip[b][32:96, lo:hi])

            # on-device dw0 block: center globally shifted +1 element.
            # quarter q copies src [32:64, lo:hi) -> dst [0:32, lo+1:hi+1)
            # DVE takes 3 quarters (hits 4x perf mode), ACT takes 1 (1x).
            def emit_dw0_quarter(b, q):
                slab = slabs[b]
                lo, hi = QRANGE[q]
                if q == 3:
                    hi -= 2              # keep count even; tail never read
                src = slab[32:64, lo:hi]
                dst = slab[0:32, lo + 1:hi + 1]
                if q == 3:
                    nc.scalar.copy(out=dst, in_=src)
                else:
                    nc.vector.tensor_copy(out=dst, in_=src)

            # ---- hypernetwork: Wk = P @ dense_w (permuted), 4 chunks ----
            w_sb = wsb_pool.tile([QK, BPC * G], bf16, name="w_sb", tag="w_sb")
            with tc.tile_pool(name="wps", bufs=4, space="PSUM") as wps_pool:
                for chunk in range(4):
                    wps = wps_pool.tile([QK, 24 * BPC], f32, name="wps",
                                        tag="wps")
                    for gg in range(24):
                        g = chunk * 24 + gg      # g = dh_rev*C_OUT + co
                        nc.tensor.matmul(
                            out=wps[:, gg * BPC:(gg + 1) * BPC],
                            lhsT=dwsb[:, g * QK:(g + 1) * QK],
                            rhs=p_sb[:],
                            start=True, stop=True,
                        )
                    # permute (g, b) -> (b, g) while casting f32 -> bf16
                    src = wps[:].rearrange("p (g b) -> p g b", b=BPC)
                    dst = w_sb[:].rearrange("p (b g) -> p g b", g=G)[
                        :, chunk * 24:(chunk + 1) * 24, :]
                    nc.vector.tensor_copy(out=dst, in_=src)

            nc.gpsimd.memset(slabs[0][0:32, 0:1], 0.0)
            for q in range(4):
                emit_dw0_quarter(0, q)

            # ---- per-sample conv ----
            with tc.tile_pool(name="acc", bufs=6, space="PSUM") as acc_pool:
                for b in range(BPC):
                    if b + 1 < BPC:
                        nc.gpsimd.memset(slabs[b + 1][0:32, 0:1], 0.0)
                        for q in range(4):
                            emit_dw0_quarter(b + 1, q)
                    slab = slabs[b]
                    osb = None
                    tiles = {}      # t -> psum AP [W, RPT*C_OUT]
                    for hp in range(H2):
                        lhsT = slab[:, hp * W2: hp * W2 + W]
                        rows = [r for r in (hp - 2, hp - 1, hp)
                                if 0 <= r < H]
                        groups = []
                        for r in rows:
                            t = r // RPT
                            if groups and groups[-1][0] == t:
                                groups[-1][1].append(r)
                            else:
                                groups.append((t, [r]))
                        for t, rs in groups:
                            if t not in tiles:
                                tiles[t] = acc_pool.tile(
                                    [W, RPT * C_OUT], f32, name="acc",
                                    tag="acc")
                            r_lo, r_hi = rs[0], rs[-1]
                            c_lo = r_lo % RPT
                            w_lo = 2 - (hp - r_lo)
                            nc.tensor.matmul(
                                out=tiles[t][:, c_lo * C_OUT:
                                             (c_lo + len(rs)) * C_OUT],
                                lhsT=lhsT,
                                rhs=w_sb[:, b * G + w_lo * C_OUT:
                                         b * G + (w_lo + len(rs)) * C_OUT],
                                start=(r_lo % RPT == 0 and hp - r_lo == 0),
                                stop=(r_hi % RPT == RPT - 1
                                      and hp - r_hi == 2),
                                skip_group_check=True,
                            )
                        t_done = None
                        if hp >= 2 and (hp - 2) % RPT == RPT - 1:
                            t_done = (hp - 2) // RPT
                        if t_done is not None:
                            gpos = t_done % 4
                            if gpos == 0:
                                osb = osb_pool.tile([W, 4 * RPT * C_OUT],
                                                    bf16, name="osb",
                                                    tag="osb")
                            src = tiles.pop(t_done)
                            dst = osb[:, gpos * RPT * C_OUT:
                                      (gpos + 1) * RPT * C_OUT]
                            if t_done % 2 == 0:
                                nc.vector.tensor_copy(out=dst, in_=src[:])
                            else:
                                nc.scalar.copy(out=dst, in_=src[:])
                            if b == BPC - 1 and gpos in (1, 3):
                                g0 = (t_done - 1) * RPT * C_OUT
                                o0 = (gpos - 1) * RPT * C_OUT
                                nc.gpsimd.dma_start(
                                    out=y[b][:, g0:g0 + 2 * RPT * C_OUT],
                                    in_=osb[:, o0:o0 + 2 * RPT * C_OUT],
                                )
                            elif b < BPC - 1 and gpos == 3:
                                g0 = (t_done - 3) * RPT * C_OUT
                                nc.gpsimd.dma_start(
                                    out=y[b][:, g0:g0 + 4 * RPT * C_OUT],
                                    in_=osb[:],
                                )
    nc.finalize()
    return nc


def _get_nc():
    if "nc" not in _NC_CACHE:
        _NC_CACHE["nc"] = _build_nc()
    return _NC_CACHE["nc"]


def _prep_inputs(X, P, dense_w):
    bf16 = ml_dtypes.bfloat16
    Xb = np.ascontiguousarray(X.transpose(0, 3, 1, 2)).astype(bf16)  # [B,C,H,W]
    X_trip = np.zeros((B, QK, H2, W2), dtype=bf16)
    for dw in range(KW):
        lo = max(0, 1 - dw)          # first valid wp
        hi = W - dw                  # last valid wp (inclusive)
        src_lo = lo + dw - 1
        X_trip[:, dw * C:(dw + 1) * C, 1:H + 1, lo:hi + 1] = \
            Xb[:, :, :, src_lo:W]
    X_trip = X_trip.reshape(B, QK, S)

    # dense_w columns j = ((dh*3+dw)*C+ci)*C_OUT+co -> (2-dh, co, dw, ci)
    dwp = np.ascontiguousarray(
        dense_w.reshape(P_DIM, KH, KW, C, C_OUT)[:, ::-1]
        .transpose(0, 1, 4, 2, 3)
        .reshape(P_DIM, -1)
    ).astype(bf16)

    in_maps = []
    for c in range(N_CORES):
        sl = slice(c * BPC, (c + 1) * BPC)
        in_maps.append({
            "x_trip": np.ascontiguousarray(X_trip[sl]),
            "p_t": np.ascontiguousarray(P[sl].T).astype(bf16),
            "dw_t": dwp,
        })
    return in_maps


def _run(X, P, dense_w, **spmd_kwargs):
    nc = _get_nc()
    in_maps = _prep_inputs(X, P, dense_w)
    res = run_bass_kernel_spmd(nc, in_maps, core_ids=list(range(N_CORES)),
                               **spmd_kwargs)
    outs = []
    for c in range(N_CORES):
        yv = res.results[c]["y"].astype(np.float32)
        yv = yv.reshape(BPC, W, H, C_OUT)
        outs.append(yv.transpose(0, 2, 1, 3))        # -> [b, h, w, co]
    Y = np.ascontiguousarray(np.concatenate(outs, axis=0), dtype=np.float32)
    return Y, res


def kernel(X, P, dense_w):
    Y, _ = _run(np.asarray(X), np.asarray(P), np.asarray(dense_w))
    return Y
